# revision 1
# baseline (speedup 1.0000x reference)
"""Trainium2 Bass kernel for the reset-state LSTM stack (nn_RNN_26448408609039).

Math (hidden state is reset each step, so steps are a pure batch dim):
  x = inputs @ W_in.T + b_in                      (S, 256)
  for l in 0..5:  gates = x @ W_ih[l].T + b_ih[l] + b_hh[l]
                  i,f,g,o = split(gates); c = sig(i)*tanh(g); x = sig(o)*tanh(c)
  out = x @ W_fc.T + b_fc                         (S, 63)

Device strategy (per core, steps sharded 8 ways -> 4096 steps/core):
  - Everything computed transposed: activations live as [H, steps] so the
    contraction dim sits on SBUF partitions for the tensor engine.
  - Input Linear folded into layer 0's gate matmul on the host:
      W0 = W_ih[0] @ W_in  (K=63), bias0 += W_ih[0] @ b_in.
  - h_prev is zero, so W_hh contributes nothing; f-gate is multiplied by
    c_prev = 0, so only i,g,o gate rows (768 of 1024) are computed.
  - Matmuls run in float32r (tf32) at 1 cycle/row; fp32 would be 4x slower.
  - Gate bias-add is fused into the ScalarE activation (per-partition bias AP).
"""

import numpy as np

S, D, H, L = 32768, 63, 256, 6
N_CORES = 8
SC = S // N_CORES          # steps per core
W_SC = 2048                # superchunk width (free dim of one gate phase)
N_MM = 512                 # matmul moving-operand width

_CACHE = {}


def _build():
    import concourse.bacc as bacc
    import concourse.mybir as mybir
    from concourse.tile import TileContext

    F32, F32R = mybir.dt.float32, mybir.dt.float32r
    AF = mybir.ActivationFunctionType

    nc = bacc.Bacc("TRN2", target_bir_lowering=False, debug=False, num_devices=1)
    XT = nc.dram_tensor("XT", [D, SC], F32R, kind="ExternalInput")
    W0 = nc.dram_tensor("W0", [D, 768], F32R, kind="ExternalInput")
    WL = nc.dram_tensor("WL", [L - 1, H, 768], F32R, kind="ExternalInput")
    BIAS = nc.dram_tensor("BIAS", [128, 6 * L], F32, kind="ExternalInput")
    WFC = nc.dram_tensor("WFC", [H, D], F32R, kind="ExternalInput")
    BFC = nc.dram_tensor("BFC", [D, 1], F32, kind="ExternalInput")
    OUT = nc.dram_tensor("OUT", [D, SC], F32, kind="ExternalOutput")

    with TileContext(nc) as tc:
        with tc.tile_pool(name="w", bufs=1) as wp, \
             tc.tile_pool(name="a", bufs=1) as ap, \
             tc.tile_pool(name="ps", bufs=2, space="PSUM") as ps:
            xt = wp.tile([D, SC], F32R, tag="xt")
            nc.sync.dma_start(xt[:], XT[:])
            w0 = wp.tile([D, 768], F32R, tag="w0")
            nc.sync.dma_start(w0[:], W0[:])
            wl = wp.tile([128, (L - 1) * 2 * 768], F32R, tag="wl")
            for l in range(L - 1):
                for kt in range(2):
                    nc.sync.dma_start(
                        wl[:, (l * 2 + kt) * 768:(l * 2 + kt + 1) * 768],
                        WL[l, kt * 128:(kt + 1) * 128, :])
            bias = wp.tile([128, 6 * L], F32, tag="bias")
            nc.sync.dma_start(bias[:], BIAS[:])
            wfc = wp.tile([128, 2 * D], F32R, tag="wfc")
            nc.sync.dma_start(wfc[:, 0:D], WFC[0:128, :])
            nc.sync.dma_start(wfc[:, D:2 * D], WFC[128:256, :])
            bfc = wp.tile([D, 1], F32, tag="bfc")
            nc.sync.dma_start(bfc[:], BFC[:])
            out_sb = wp.tile([D, SC], F32, tag="out_sb")

            for sc in range(SC // W_SC):
                base = sc * W_SC
                h0 = h1 = None

                for l in range(L):
                    def gate(col, func, tag):
                        # gates^T m-tile [128, W_SC]; col 0..5 = i0,i1,g0,g1,o0,o1
                        p = ps.tile([128, W_SC], F32, tag="g", name=f"p_{tag}_{sc}_{l}")
                        for ns in range(W_SC // N_MM):
                            sl = slice(ns * N_MM, (ns + 1) * N_MM)
                            if l == 0:
                                nc.tensor.matmul(
                                    p[:, sl], w0[:, col * 128:(col + 1) * 128],
                                    xt[:, base + ns * N_MM: base + (ns + 1) * N_MM],
                                    start=True, stop=True)
                            else:
                                wb = (l - 1) * 2 * 768
                                nc.tensor.matmul(
                                    p[:, sl], wl[:, wb + col * 128: wb + (col + 1) * 128],
                                    h0[:, sl], start=True, stop=False)
                                nc.tensor.matmul(
                                    p[:, sl], wl[:, wb + 768 + col * 128: wb + 768 + (col + 1) * 128],
                                    h1[:, sl], start=False, stop=True)
                        t = ap.tile([128, W_SC], F32, tag=tag, name=f"{tag}_{sc}_{l}")
                        nc.scalar.activation(t[:], p[:], func,
                                             bias=bias[:, l * 6 + col: l * 6 + col + 1])
                        return t

                    tg0 = gate(2, AF.Tanh, "tg0")
                    tg1 = gate(3, AF.Tanh, "tg1")
                    si0 = gate(0, AF.Sigmoid, "si0")
                    si1 = gate(1, AF.Sigmoid, "si1")
                    c0 = ap.tile([128, W_SC], F32, tag="c0", name=f"c0_{sc}_{l}")
                    nc.vector.tensor_mul(c0[:], si0[:], tg0[:])
                    c1 = ap.tile([128, W_SC], F32, tag="c1", name=f"c1_{sc}_{l}")
                    nc.vector.tensor_mul(c1[:], si1[:], tg1[:])
                    tc0 = ap.tile([128, W_SC], F32, tag="tc0", name=f"tc0_{sc}_{l}")
                    nc.scalar.activation(tc0[:], c0[:], AF.Tanh)
                    tc1 = ap.tile([128, W_SC], F32, tag="tc1", name=f"tc1_{sc}_{l}")
                    nc.scalar.activation(tc1[:], c1[:], AF.Tanh)
                    so0 = gate(4, AF.Sigmoid, "so0")
                    so1 = gate(5, AF.Sigmoid, "so1")
                    nh0 = ap.tile([128, W_SC], F32R, tag="h0", bufs=2, name=f"h0_{sc}_{l}")
                    nc.vector.tensor_mul(nh0[:], so0[:], tc0[:])
                    nh1 = ap.tile([128, W_SC], F32R, tag="h1", bufs=2, name=f"h1_{sc}_{l}")
                    nc.vector.tensor_mul(nh1[:], so1[:], tc1[:])
                    h0, h1 = nh0, nh1

                # final linear for this superchunk: out^T [63, W_SC]
                pf = ps.tile([D, W_SC], F32, tag="g", name=f"pf_{sc}")
                for ns in range(W_SC // N_MM):
                    sl = slice(ns * N_MM, (ns + 1) * N_MM)
                    nc.tensor.matmul(pf[:, sl], wfc[:, 0:D], h0[:, sl],
                                     start=True, stop=False)
                    nc.tensor.matmul(pf[:, sl], wfc[:, D:2 * D], h1[:, sl],
                                     start=False, stop=True)
                nc.scalar.activation(out_sb[:, base:base + W_SC], pf[:],
                                     AF.Identity, bias=bfc[:])

            nc.sync.dma_start(OUT[:], out_sb[:])
    nc.compile()
    return nc


def _prep_host(inputs, W_in, b_in, W_ih, W_hh, b_ih, b_hh, W_fc, b_fc):
    f64 = np.float64
    igo = np.r_[0:256, 512:768, 768:1024]
    # layer 0 fused with the input Linear
    W0 = (W_ih[0][igo].astype(f64) @ W_in.astype(f64))              # [768, 63]
    b0 = (b_ih[0][igo].astype(f64) + b_hh[0][igo].astype(f64)
          + W_ih[0][igo].astype(f64) @ b_in.astype(f64))            # [768]
    W0_lhsT = np.ascontiguousarray(W0.T.astype(np.float32))         # [63, 768]

    WL = np.stack([np.ascontiguousarray(W_ih[l][igo].T.astype(np.float32))
                   for l in range(1, L)])                            # [5, 256, 768]
    biases = np.zeros((128, 6 * L), np.float32)
    for l in range(L):
        if l == 0:
            b = b0
        else:
            b = b_ih[l][igo].astype(f64) + b_hh[l][igo].astype(f64)
        biases[:, l * 6:(l + 1) * 6] = b.astype(np.float32).reshape(6, 128).T

    WFC = np.ascontiguousarray(W_fc.T.astype(np.float32))            # [256, 63]
    BFC = np.ascontiguousarray(b_fc.astype(np.float32).reshape(D, 1))
    XT = np.ascontiguousarray(inputs.astype(np.float32).T)           # [63, S]
    return XT, W0_lhsT, WL, biases, WFC, BFC


def make_in_maps(inputs, W_in, b_in, W_ih, W_hh, b_ih, b_hh, W_fc, b_fc):
    XT, W0_lhsT, WL, biases, WFC, BFC = _prep_host(
        inputs, W_in, b_in, W_ih, W_hh, b_ih, b_hh, W_fc, b_fc)
    in_maps = []
    for c in range(N_CORES):
        in_maps.append({
            "XT": np.ascontiguousarray(XT[:, c * SC:(c + 1) * SC]),
            "W0": W0_lhsT, "WL": WL, "BIAS": biases, "WFC": WFC, "BFC": BFC,
        })
    return in_maps


def get_nc():
    if "nc" not in _CACHE:
        _CACHE["nc"] = _build()
    return _CACHE["nc"]


def assemble(results):
    outT = np.concatenate([results[c]["OUT"] for c in range(N_CORES)], axis=1)
    return np.ascontiguousarray(outT.T.astype(np.float32))[:, None, :]


def kernel(inputs, W_in, b_in, W_ih, W_hh, b_ih, b_hh, W_fc, b_fc):
    from concourse.bass_utils import run_bass_kernel_spmd
    nc = get_nc()
    in_maps = make_in_maps(inputs, W_in, b_in, W_ih, W_hh, b_ih, b_hh, W_fc, b_fc)
    res = run_bass_kernel_spmd(nc, in_maps, list(range(N_CORES)))
    return assemble(res.results)
